# revision 1
# baseline (speedup 1.0000x reference)
"""GAT (4-layer, softmax over dim=1) Trainium2 Bass kernel.

Sharding: data-parallel over batch B=8 -> one batch element per NeuronCore,
zero collectives. ~290 us HW exec (from 364 us), rel err ~1.2e-3.

Core rewrite vs the previous version: the attention exponentials are computed
in EXP DOMAIN as rank-1 products, which deletes both N^2 ACT passes
(Prelu + Exp) of the old kernel:

    exp(leakyrelu(f1[i]+f2[j])) = max(exp(f1)exp(f2), exp(.2f1)exp(.2f2))

and, since softmax over dim=1 (i) is invariant to any per-column (j) scale,
the whole column is divided by exp(f2[j]): only d[j] = exp(-0.8 f2[j])
survives, and the removed factor cancels inside whs = Wh/s'. One full [N,N]
product pass disappears outright.

Per j-strip [128 x 2048] in T layout (j on partitions, so the dim=1 softmax
reduce is a free-axis accumulate and the PE out-matmul takes E directly):
    DVE TS (4x): A2 = v2b * d[j]          v2b = exp(.2 f1) broadcast
    DVE TT (2x): M  = max(v1b, A2)        v1b = exp(f1) broadcast
    DVE TT (2x): Em = M * mask01          ({0,1} f16 mask)
    ACT Copy+accum_out -> s'[j]           free-axis reduce; the copy output
                                          is a dead fp8 scratch (ACT is
                                          io-byte bound); strip 15 reduces
                                          on DVE to shorten the layer drain
    DVE: r = 1/s' (per strip pair)
    whs = Wh * r[j] (alternating DVE TS / ACT Copy-scale) -> f16
    PE : outT[o,i] += whs.T @ Em          accumulated in 8 psum banks

Per-layer 2^m scaling (WHS_M) keeps whs in f16 normal range: W is pre-scaled
2^(m_l - m_{l-1}) on host, hT carries 2^m_l, exp scale immediates divide it
back out, and the host undoes 2^m_3 on the final output. Final [o,i]->[i,o]
transpose happens on host (output DMA'd in T layout).

Measured HW notes: DVE dual-op TensorScalar runs 4x (0.26ns/col), TensorTensor
2x (0.52ns/col), STT/reduce/TS-with-accum 1x; fast modes need 2-byte packed
operands everywhere (fp8/f32 kill them). GpSimd has no TT/TS ISA. fp8 is
unusable on the Wh side regardless of scaling: the output is a zero-mean dot
over whs, so whs quantization error does not average out (E-side fp8 errors
do cancel via the softmax, but buy no speed here since DoubleRow needs both
operands fp8). Walrus accepts only one sync-wait per instruction
(split_multi_waits). ACT activation computes func(scale*x + bias) with
[128,1] AP scale/bias; Copy supports accum_out = the cheapest free-axis
reduce on the machine.
"""
import numpy as np
import ml_dtypes

import bass_rust
import concourse.bass as bass
import concourse.mybir as mybir
import concourse.tile as tile
from concourse.bass_utils import run_bass_kernel_spmd

f32 = mybir.dt.float32
bf16 = mybir.dt.bfloat16
f16 = mybir.dt.float16
AFT = mybir.ActivationFunctionType
ALU = mybir.AluOpType

B, N, F, L = 8, 2048, 256, 4
NT = N // 128   # 16 j-strips
FC = F // 128   # 2 feature chunks
IC = N // 512   # 4 i-chunks per strip
NQ = NT // 4    # 4 strip-quads
ALPHA = 0.2
# per-layer whs scale exponents: whs*2^m lands ~[0.1, 8] in f16 normal range
WHS_M = [9, 13, 15, 16]


def split_multi_waits(nc):
    """This container's walrus supports at most one sync-wait per instruction;
    Tile's exit drain (and occasionally the scheduler) attaches several. Hoist
    extras onto same-engine EventSemaphore instructions placed just before."""
    for fn in nc.m.functions:
        for blk in fn.blocks:
            new_list, changed = [], False
            for inst in blk.instructions:
                si = inst.sync_info
                if si is not None and len(si.on_wait) > 1:
                    waits = list(si.on_wait)
                    for k, w in enumerate(waits[:-1]):
                        es = mybir.InstEventSemaphore(name=f"{inst.name}_wsplit{k}")
                        es.engine = inst.engine
                        es.sync_info = bass_rust.SyncInfo(on_wait=[w], on_update=[])
                        new_list.append(es)
                    si.on_wait = [waits[-1]]
                    changed = True
                new_list.append(inst)
            if changed:
                blk.instructions = new_list


def build_nc(do_split=True):
    nc = bass.Bass()
    xT_d = nc.dram_tensor("xT", [F, N], f16, kind="ExternalInput")
    mask_d = nc.dram_tensor("maskT", [N, N], f16, kind="ExternalInput")
    W_d = nc.dram_tensor("W", [L, F, F], f16, kind="ExternalInput")
    wa2_d = nc.dram_tensor("wa2", [L, F, 1], f16, kind="ExternalInput")  # W@a2
    wab_d = nc.dram_tensor("wab", [L, F, 128], f16, kind="ExternalInput")  # W@a1
    ident_d = nc.dram_tensor("ident", [128, 128], f16, kind="ExternalInput")
    out_d = nc.dram_tensor("out", [F, N], f16, kind="ExternalOutput")

    with tile.TileContext(nc) as tc:
        with (
            tc.tile_pool(name="const", bufs=1) as constp,
            tc.tile_pool(name="hT", bufs=2) as hTp,
            tc.tile_pool(name="wl", bufs=2) as wlp,
            tc.tile_pool(name="wh", bufs=1) as whp,
            tc.tile_pool(name="vb", bufs=1) as vbp,
            tc.tile_pool(name="cs", bufs=2) as csp,
            tc.tile_pool(name="aq", bufs=1) as aqp,
            tc.tile_pool(name="em", bufs=5) as emp,
            tc.tile_pool(name="ws", bufs=6) as wsp,
            tc.tile_pool(name="sr", bufs=4) as srp,
            tc.tile_pool(name="outsb", bufs=3) as outp,
            tc.tile_pool(name="bank", bufs=8, space="PSUM") as psp,
        ):
            ident_sb = constp.tile([128, 128], f16)
            nc.sync.dma_start(ident_sb[:], ident_d[:])
            rscratch = constp.tile([128, N], mybir.dt.float8e4)  # dead reduce output (fp8: ACT is io-byte bound)

            hT_cur = hTp.tile([128, FC * N], f16, tag="hT")
            for fc in range(FC):
                for h4 in range(4):
                    nc.sync.dma_start(
                        hT_cur[:, fc * N + h4 * 512 : fc * N + (h4 + 1) * 512],
                        xT_d[fc * 128 : (fc + 1) * 128, h4 * 512 : (h4 + 1) * 512],
                    )

            def load_layer_weights(l):
                W_sb = wlp.tile([128, FC * F], f16, tag="W", name=f"W_{l}")
                wa2_sb = wlp.tile([128, FC * 1], f16, tag="wa2", name=f"wa2_{l}")
                wab_sb = wlp.tile([128, FC * 128], f16, tag="wab", name=f"wab_{l}")
                for fc in range(FC):
                    nc.sync.dma_start(
                        W_sb[:, fc * F : (fc + 1) * F],
                        W_d[l, fc * 128 : (fc + 1) * 128, :],
                    )
                    nc.sync.dma_start(
                        wa2_sb[:, fc : fc + 1],
                        wa2_d[l, fc * 128 : (fc + 1) * 128, :],
                    )
                    nc.sync.dma_start(
                        wab_sb[:, fc * 128 : (fc + 1) * 128],
                        wab_d[l, fc * 128 : (fc + 1) * 128, :],
                    )
                return W_sb, wa2_sb, wab_sb

            weights0 = load_layer_weights(0)
            mask_sb = constp.tile([128, NT * N], f16)
            for jt in range(NT):
                nc.sync.dma_start(
                    mask_sb[:, jt * N : (jt + 1) * N],
                    mask_d[jt * 128 : (jt + 1) * 128, :],
                )

            for l in range(L):
                if l == 0:
                    W_sb, wa2_sb, wab_sb = weights0
                else:
                    W_sb, wa2_sb, wab_sb = load_layer_weights(l)

                # ---- f-phase ----
                # f2 for all strips into one psum bank [128,16]; c = exp(f2)
                # softmax over i is invariant to per-column scale: divide
                # column j by exp(f2[j]); only d = exp(-0.8 f2) survives, and
                # the 1/exp(f2) factor cancels inside whs = Wh/s'.
                uprev = float(2.0 ** (-WHS_M[l - 1])) if l > 0 else 1.0
                ps_f2 = psp.tile([128, 512], f32, tag="bank", name=f"psf2_{l}")
                d_sb = csp.tile([128, NT], f32, tag="d", name=f"d_{l}")
                for g in range(4):
                    for nt in range(4 * g, 4 * g + 4):
                        for fc in range(FC):
                            nc.tensor.matmul(
                                ps_f2[:, nt : nt + 1],
                                hT_cur[:, fc * N + nt * 128 : fc * N + (nt + 1) * 128],
                                wa2_sb[:, fc : fc + 1],
                                start=(fc == 0),
                                stop=(fc == FC - 1),
                            )
                    nc.scalar.activation(
                        d_sb[:, 4 * g : 4 * g + 4], ps_f2[:, 4 * g : 4 * g + 4],
                        AFT.Exp, scale=-0.8 * uprev,
                    )

                # f1 broadcast chunks; v2b = exp(0.2 f1) first (strip 0's TS
                # needs it before v1b is needed by the max)
                v1b = vbp.tile([128, N], f16, tag="v1b", name=f"v1b_{l}")
                v2b = vbp.tile([128, N], f16, tag="v2b", name=f"v2b_{l}")
                f1ps = []
                for ic in range(IC):
                    ps = psp.tile([128, 512], f32, tag="bank", name=f"psf1b_{l}_{ic}")
                    for fc in range(FC):
                        nc.tensor.matmul(
                            ps[:, :],
                            wab_sb[:, fc * 128 : (fc + 1) * 128],
                            hT_cur[:, fc * N + ic * 512 : fc * N + (ic + 1) * 512],
                            start=(fc == 0),
                            stop=(fc == FC - 1),
                        )
                    f1ps.append(ps)
                    nc.scalar.activation(
                        v2b[:, ic * 512 : (ic + 1) * 512], ps[:, :], AFT.Exp,
                        scale=ALPHA * uprev,
                    )
                for ic in range(IC):
                    nc.scalar.activation(
                        v1b[:, ic * 512 : (ic + 1) * 512], f1ps[ic][:, :], AFT.Exp,
                        scale=uprev,
                    )

                # Wh = hT.T @ W -> SBUF bf16 (PE continues while DVE starts)
                Wh_sb = whp.tile([128, NT * F], f16, tag="Wh")
                for nt in range(NT):
                    ps = psp.tile([128, 512], f32, tag="bank", name=f"psWh_{l}_{nt}")
                    for fc in range(FC):
                        nc.tensor.matmul(
                            ps[:, 0:F],
                            hT_cur[:, fc * N + nt * 128 : fc * N + (nt + 1) * 128],
                            W_sb[:, fc * F : (fc + 1) * F],
                            start=(fc == 0),
                            stop=(fc == FC - 1),
                        )
                    nc.scalar.copy(Wh_sb[:, nt * F : (nt + 1) * F], ps[:, 0:F])

                # ---- strip loop: per-strip dribble, pair-level finalize ----
                psum_out = [
                    psp.tile([128, 512], f32, tag="bank", name=f"po_{l}_{k}")
                    for k in range(8)
                ]
                em_t = [None] * NT
                s_t = [None] * (NT // 2)
                ws_t = [None] * NT

                def emit_strip(jt):
                    a2 = aqp.tile([128, N], f16, tag="a2", name=f"a2_{l}_{jt}")
                    m = aqp.tile([128, N], f16, tag="m", name=f"m_{l}_{jt}")
                    em = emp.tile([128, N], f16, tag="em", name=f"em_{l}_{jt}")
                    em_t[jt] = em
                    nh = 2 if jt < 2 else 1
                    for c in range(nh):
                        lo, hi = c * N // nh, (c + 1) * N // nh
                        nc.vector.tensor_scalar_mul(
                            a2[:, lo:hi], v2b[:, lo:hi], d_sb[:, jt : jt + 1]
                        )
                        nc.vector.tensor_tensor(
                            m[:, lo:hi], v1b[:, lo:hi], a2[:, lo:hi], ALU.max
                        )
                        nc.vector.tensor_tensor(
                            em[:, lo:hi], m[:, lo:hi],
                            mask_sb[:, jt * N + lo : jt * N + hi], ALU.mult,
                        )
                    p, k = jt // 2, jt % 2
                    if k == 0:
                        s_t[p] = srp.tile([128, 2], f32, tag="s", name=f"s_{l}_{p}")
                    if jt == NT - 1:
                        nc.vector.tensor_reduce(
                            s_t[p][:, k : k + 1], em[:, :],
                            bass_rust.AxisListType.X, ALU.add,
                        )
                    else:
                        nc.scalar.activation(
                            rscratch[:, :], em[:, :], AFT.Copy,
                            accum_out=s_t[p][:, k : k + 1],
                        )

                def finalize_pair(p, last):
                    r_p = srp.tile([128, 2], f32, tag="r", name=f"r_{l}_{p}")
                    nc.vector.reciprocal(r_p[:, :], s_t[p][:, :])
                    for k in range(2):
                        jt = 2 * p + k
                        w = wsp.tile([128, F], f16, tag="ws", name=f"ws_{l}_{jt}")
                        ws_t[jt] = w
                        if jt % 2 == 0:
                            nc.vector.tensor_scalar_mul(
                                w[:, :], Wh_sb[:, jt * F : (jt + 1) * F],
                                r_p[:, k : k + 1],
                            )
                        else:
                            nc.scalar.activation(
                                w[:, :], Wh_sb[:, jt * F : (jt + 1) * F],
                                AFT.Copy, scale=r_p[:, k : k + 1],
                            )
                        em = em_t[jt]
                        for oc in range(FC):
                            for ic in range(IC):
                                nc.tensor.matmul(
                                    psum_out[oc * IC + ic][:, :],
                                    w[:, oc * 128 : (oc + 1) * 128],
                                    em[:, ic * 512 : (ic + 1) * 512],
                                    start=(jt == 0),
                                    stop=(last and jt == NT - 1),
                                )

                for jt in range(NT):
                    emit_strip(jt)
                    if jt >= 3 and jt % 2 == 1:
                        finalize_pair((jt - 3) // 2, last=False)
                finalize_pair(NT // 2 - 1, last=True)

                # ---- tail: hT_next = prelu(outT); carries 2^m_l, undone by
                # the next layer's exp scale immediates / host at the end ----
                hT_next = hTp.tile([128, FC * N], f16, tag="hT")
                for ic in range(IC):
                    for oc in range(FC):
                        dst = hT_next[:, oc * N + ic * 512 : oc * N + (ic + 1) * 512]
                        ps = psum_out[oc * IC + ic]
                        nc.scalar.activation(dst, ps[:, :], AFT.Prelu, alpha=ALPHA)
                if l < L - 1:
                    hT_cur = hT_next
                else:
                    # out stays in [o, i] layout; host transposes
                    for fc in range(FC):
                        nc.sync.dma_start(
                            out_d[fc * 128 : (fc + 1) * 128, :],
                            hT_next[:, fc * N : (fc + 1) * N],
                        )

    if do_split:
        split_multi_waits(nc)
    return nc


_NC = None


def _get_nc():
    global _NC
    if _NC is None:
        _NC = build_nc()
    return _NC


def _host_prep(x, adj, W0, Wrest, A):
    x = np.asarray(x, dtype=np.float32)
    adj = np.asarray(adj)
    W_all = np.stack(
        [np.asarray(W0, dtype=np.float32)]
        + [np.asarray(Wrest[i], dtype=np.float32) for i in range(L - 1)]
    )
    A = np.asarray(A, dtype=np.float32)
    wa2 = np.empty((L, F, 1), dtype=np.float32)
    wab = np.empty((L, F, 128), dtype=np.float32)
    for l in range(L):
        # fold the per-layer whs 2^m into the c-exponent? No: c scales E and s
        # identically (cancels in softmax); the 2^m rides on whs via the ACT
        # scale being r*... -- simplest is scaling Wh itself here.
        wa2[l, :, 0] = W_all[l] @ A[l, F:]
        wab[l] = np.repeat((W_all[l] @ A[l, :F])[:, None], 128, axis=1)
    ident = np.eye(128, dtype=np.float16)
    # W_eff[l] = W[l] * 2^(m_l - m_{l-1}); hT then carries 2^{m_l} through
    # the layer, divided out by exp-scale immediates and the final host step
    W_16 = np.stack(
        [
            (W_all[l] * (2.0 ** (WHS_M[l] - (WHS_M[l - 1] if l > 0 else 0))))
            .astype(np.float16)
            for l in range(L)
        ]
    )
    wa2_16 = wa2.astype(np.float16)
    wab_16 = wab.astype(np.float16)

    in_maps = []
    for b in range(B):
        xT = np.ascontiguousarray(x[b].T).astype(np.float16)
        maskT = adj[b].T.astype(np.float16)
        in_maps.append(
            {
                "xT": xT,
                "maskT": maskT,
                "W": W_16,
                "wa2": wa2_16,
                "wab": wab_16,
                "ident": ident,
            }
        )
    return in_maps


def kernel(x, adj, W0, Wrest, A, _trace=False, _trace_kwargs=None):
    nc = _get_nc()
    in_maps = _host_prep(x, adj, W0, Wrest, A)
    res = run_bass_kernel_spmd(
        nc,
        in_maps,
        core_ids=list(range(B)),
        trace=_trace,
        **(_trace_kwargs or {}),
    )
    unscale = np.float32(2.0 ** (-WHS_M[-1]))
    out = np.stack(
        [res.results[b]["out"].astype(np.float32).T * unscale for b in range(B)]
    )
    if _trace:
        kernel.last_exec_time_ns = res.exec_time_ns
        kernel.last_results = res
    return out

